# revision 45
# baseline (speedup 1.0000x reference)
"""Trainium2 Bass kernel for nn_MultiHeadAttention_377957122345.

B=16, T=512, C=1024, H=16, D=64.  Data-parallel over batch: each of the
8 NeuronCores computes attention for 2 sequences; no collectives.

v2 design (PE-continuity + DMA-count oriented):
  - all inputs staged host-side in bf16 (x transposed, W^T, exp(bias)
    with causal mask folded in and packed per head into one [128,1280]
    row-block so each head needs a single DMA).
  - projections: 16 groups of 8 matmuls per tensor, PSUM [128,512].
  - attention is software-pipelined 2 deep per head:
      S (4 matmuls, B-half first) -> exp (2 instrs) -> bias mul (DVE 2x)
      -> AV (4 accumulating matmuls into one [65,512] PSUM bank, ones
      column yields the softmax denominator as row 64) -> reciprocal
      -> rank-1 PE broadcast of 1/l -> fused normalize+copy into oT.
  - V (2nd sequence) and output-projection matmul groups are interleaved
    into the attention stream as PE gap fillers so the Tensor engine
    never idles (TRN2 PE drops to 1.2GHz after any stall).
  - PSUM: psA 2 banks + psB 1 + psP 2 + psO 3 = 8 banks exactly.
"""

import contextlib

import numpy as np

B, T, C, H = 16, 512, 1024, 16
D = C // H  # 64
N_CORES = 8
B_LOC = B // N_CORES  # 2 sequences per core
NT = B_LOC * T  # 1024 tokens per core
P = 128
KO = C // P  # 8 contraction subtiles
TB = T // P  # 4 query blocks per sequence
NEG = -1e30
BWID = 1280  # packed bias row: 512 + 384 + 256 + 128

_CACHE = {}


def _nullcm():
    return contextlib.nullcontext()


def _split_big_waits(nc, mybir, limit=1):
    # This walrus build rejects instructions whose sync_info.on_wait
    # exceeds its slot count (the Tile end-of-kernel Drain trips it).
    # Move excess waits onto dedicated same-engine NOPs placed directly
    # before the instruction; the engine stalls on those first, so the
    # semantics are unchanged.
    f = nc.m.functions[0]
    for bb in f.blocks:
        out = []
        changed = False
        for inst in bb.instructions:
            si = getattr(inst, "sync_info", None)
            waits = list(si.on_wait) if si is not None and si.on_wait else []
            if len(waits) > limit:
                changed = True
                head, tail = waits[:-limit], waits[-limit:]
                for k in range(0, len(head), limit):
                    out.append(
                        mybir.InstNoOp(
                            name=f"I-{nc.next_id()}",
                            sync_info=mybir.SyncInfo(
                                on_wait=head[k : k + limit], on_update=[]
                            ),
                            bass_nofuse=True,
                            engine=inst.engine,
                        )
                    )
                si.on_wait = tail
            out.append(inst)
        if changed:
            bb.instructions = out
    return nc


class _Filler:
    """Paced stream of deferred instruction-emitting generators.

    Each unit is (gate, generator): the generator yields once per emitted
    matmul; trailing non-PE ops are emitted just before StopIteration.
    Units are consumed in order; a unit whose gate exceeds the current
    pipeline step blocks the stream until the step reaches it.
    """

    def __init__(self, units):
        self.units = list(units)
        self.pos = 0
        self.cur = None

    def emit(self, step, budget):
        while budget > 0:
            if self.cur is None:
                if self.pos >= len(self.units):
                    return
                gate, factory = self.units[self.pos]
                if gate > step:
                    return
                self.cur = factory()
                self.pos += 1
            try:
                while budget > 0:
                    next(self.cur)
                    budget -= 1
            except StopIteration:
                self.cur = None

    def drain_gated_at_most(self, step):
        # emit everything whose gate is <= step
        self.emit(step, 1 << 30)


def build_program(split_waits=True, reps=1, fill_per_head=4,
                  skip_attn=False, skip_proj=False, unroll=1):
    import concourse.bass as bass
    import concourse.mybir as mybir
    import concourse.tile as tile

    fp32 = mybir.dt.float32
    bf16 = mybir.dt.bfloat16
    Act = mybir.ActivationFunctionType

    nc = bass.Bass()
    xT = nc.dram_tensor("xT", [C, NT], bf16, kind="ExternalInput")
    wqT = nc.dram_tensor("wqT", [C, C], bf16, kind="ExternalInput")
    wkT = nc.dram_tensor("wkT", [C, C], bf16, kind="ExternalInput")
    wvT = nc.dram_tensor("wvT", [C, C], bf16, kind="ExternalInput")
    woT = nc.dram_tensor("woT", [C, C], bf16, kind="ExternalInput")
    bqd = nc.dram_tensor("bq", [C], fp32, kind="ExternalInput")
    bkd = nc.dram_tensor("bk", [C], fp32, kind="ExternalInput")
    bvd = nc.dram_tensor("bvb", [C], bf16, kind="ExternalInput")
    bod = nc.dram_tensor("bob", [C], bf16, kind="ExternalInput")
    bmp = nc.dram_tensor("bmp", [H, P, BWID], bf16, kind="ExternalInput")
    y = nc.dram_tensor("y", [NT, C], bf16, kind="ExternalOutput")
    r_dram = nc.dram_tensor("r_scratch", [B_LOC, H, T], bf16)

    with tile.TileContext(nc) as tc, \
         tc.tile_pool(name="consts", bufs=1) as consts, \
         tc.tile_pool(name="persist", bufs=1) as persist, \
         tc.tile_pool(name="biasp", bufs=3) as biasp, \
         tc.tile_pool(name="soft", bufs=2) as soft, \
         tc.tile_pool(name="small", bufs=3) as small, \
         tc.tile_pool(name="ypool", bufs=2) as ypool, \
         tc.tile_pool(name="psA", bufs=1, space="PSUM") as psA, \
         tc.tile_pool(name="psP", bufs=3, space="PSUM") as psP, \
         tc.tile_pool(name="psO", bufs=2, space="PSUM") as psO, \
         (tc.For_i(0, reps, 1) if reps > 1 else _nullcm()):
     for _unroll_i in range(unroll):
        # ----- constants -----
        bq_sb = consts.tile([P, KO], fp32, name="bq_sb")
        nc.sync.dma_start(out=bq_sb, in_=bqd.rearrange("(o p) -> p o", p=P))
        bk_sb = consts.tile([P, KO], fp32, name="bk_sb")
        nc.sync.dma_start(out=bk_sb, in_=bkd.rearrange("(o p) -> p o", p=P))
        bv_row = consts.tile([1, C], bf16, name="bv_row")
        bv_ap = bvd[:]
        nc.sync.dma_start(
            out=bv_row,
            in_=bass.AP(tensor=bv_ap.tensor, offset=bv_ap.offset, ap=[[0, 1], [1, C]]),
        )
        bo_row = consts.tile([1, C], bf16, name="bo_row")
        bo_ap = bod[:]
        nc.sync.dma_start(
            out=bo_row,
            in_=bass.AP(tensor=bo_ap.tensor, offset=bo_ap.offset, ap=[[0, 1], [1, C]]),
        )
        ones1 = consts.tile([1, P], bf16, name="ones1")
        nc.vector.memset(ones1[:], 1.0)

        # ----- persistent layouts -----
        xT_bf = persist.tile([P, KO, NT], bf16, name="xT_bf")
        wq_bf = persist.tile([P, KO, C], bf16, name="wq_bf")
        wk_bf = persist.tile([P, KO, C], bf16, name="wk_bf")
        wv_bf = persist.tile([P, KO, C], bf16, name="wv_bf")
        wo_bf = persist.tile([P, KO, C], bf16, name="wo_bf")
        qT_bf = persist.tile([P, KO, NT], bf16, name="qT_bf")
        kT_bf = persist.tile([P, KO, NT], bf16, name="kT_bf")
        vaug = persist.tile([P, NT // P, H * (D + 1)], bf16, name="vaug")
        oT_bf = persist.tile([P, KO, NT], bf16, name="oT_bf")

        def load2(dst, dram, n):
            # [C, n] dram -> [P, KO, n] sbuf in two DMAs of 4 row-blocks
            d_ap = dram[:]
            for k in range(2):
                nc.sync.dma_start(
                    out=dst[:, k * 4 : (k + 1) * 4, :],
                    in_=bass.AP(
                        tensor=d_ap.tensor,
                        offset=d_ap.offset + k * 4 * P * n,
                        ap=[[n, P], [P * n, 4], [1, n]],
                    ),
                )

        load2(wq_bf, wqT, C)
        load2(xT_bf, xT, NT)
        load2(wk_bf, wkT, C)
        load2(wv_bf, wvT, C)
        load2(wo_bf, woT, C)

        # ones columns of vaug (col h*65+64 per head); values are written
        # by the V-projection adds and never touch these columns.
        va = vaug[:]
        nc.gpsimd.memset(
            bass.AP(
                tensor=va.tensor,
                offset=va.offset + D,
                ap=[va.ap[0], [H * (D + 1), NT // P], [D + 1, H]],
            ),
            1.0,
        )

        # ----- projection group emitters -----
        def qk_group(w_bf, out_bf, b_sb, mo, nch):
            ps = psP.tile([P, 512], fp32, tag="psP")
            for ko in range(KO):
                nc.tensor.matmul(
                    ps,
                    lhsT=w_bf[:, ko, mo * P : (mo + 1) * P],
                    rhs=xT_bf[:, ko, nch * 512 : (nch + 1) * 512],
                    start=(ko == 0),
                    stop=(ko == KO - 1),
                )
            nc.scalar.activation(
                out=out_bf[:, mo, nch * 512 : (nch + 1) * 512],
                in_=ps,
                func=Act.Identity,
                bias=b_sb[:, mo : mo + 1],
            )

        def v_group(to, nch):
            def g():
                ps = psP.tile([P, 512], fp32, tag="psP")
                for ko in range(KO):
                    nc.tensor.matmul(
                        ps,
                        lhsT=xT_bf[:, ko, to * P : (to + 1) * P],
                        rhs=wv_bf[:, ko, nch * 512 : (nch + 1) * 512],
                        start=(ko == 0),
                        stop=False,
                    )
                    yield
                # rank-1 bias add inside the accumulation: ones x bv_row
                nc.tensor.matmul(
                    ps,
                    lhsT=ones1[0:1, :],
                    rhs=bv_row[0:1, nch * 512 : (nch + 1) * 512],
                    start=False,
                    stop=True,
                )
                # strided copy into vaug: 8 heads x 64 cols (stride 65)
                t_ap = vaug[:, to, :]
                outap = bass.AP(
                    tensor=t_ap.tensor,
                    offset=t_ap.offset + nch * 8 * (D + 1),
                    ap=[t_ap.ap[0], [D + 1, 8], [1, D]],
                )
                nc.scalar.activation(out=outap, in_=ps, func=Act.Copy)
            return g

        def o_pair(to):
            def g():
                ysb = ypool.tile([P, C], bf16, tag="ysb")
                for nch in range(2):
                    ps = psP.tile([P, 512], fp32, tag="psP")
                    for co in range(KO):
                        nc.tensor.matmul(
                            ps,
                            lhsT=oT_bf[:, co, to * P : (to + 1) * P],
                            rhs=wo_bf[:, co, nch * 512 : (nch + 1) * 512],
                            start=(co == 0),
                            stop=False,
                        )
                        yield
                    nc.tensor.matmul(
                        ps,
                        lhsT=ones1[0:1, :],
                        rhs=bo_row[0:1, nch * 512 : (nch + 1) * 512],
                        start=False,
                        stop=True,
                    )
                    if nch == 0:
                        nc.scalar.activation(
                            out=ysb[:, 0:512], in_=ps, func=Act.Copy
                        )
                    else:
                        nc.vector.tensor_copy(out=ysb[:, 512:1024], in_=ps)
                # issue from the Act queue so the SP queue (bias + next
                # iteration's weight loads) is never blocked behind y waits
                nc.scalar.dma_start(out=y[to * P : (to + 1) * P, :], in_=ysb)
            return g

        # ----- phase 1: Q, K projections and V for sequence 0 -----
        if not skip_proj:
            for mo in range(KO):
                for nch in range(2):
                    qk_group(wq_bf, qT_bf, bq_sb, mo, nch)
            for mo in range(KO):
                for nch in range(2):
                    qk_group(wk_bf, kT_bf, bk_sb, mo, nch)
            for to in range(2 * TB):
                for nch in range(2):
                    for _ in v_group(to, nch)():
                        pass
        else:
            nc.vector.memset(qT_bf[:], 0.01)
            nc.vector.memset(kT_bf[:], 0.01)

        # b-inner head order: each bias pair tile serves 4 consecutive
        # steps (2 heads x 2 sequences); attention runs as its own phase
        # (small PE stream pays mid p-state, the big projection streams
        # stay continuous at full clock); O-projection is a clean tail.
        heads = [(b, h) for h in range(H) for b in range(B_LOC)]
        NHEADS = len(heads)
        units = [(NHEADS + 2, o_pair(to)) for to in range(2 * TB)]
        fill = _Filler(units)

        # ----- attention pipeline -----
        stage = {}

        bt2_cur = {}

        def emit_S(i):
            b, h = heads[i]
            mo, po = h // 2, (h % 2) * D
            qh = qT_bf[po : po + D, mo, b * T : (b + 1) * T]
            kh = kT_bf[po : po + D, mo, b * T : (b + 1) * T]
            if h % 2 == 0 and b == 0:
                # one DMA covers this head pair's packed bias rows, reused
                # across both sequences (4 consecutive pipeline steps)
                bt2 = biasp.tile([P, 2 * BWID], bf16, tag="bias")
                h_ap = bmp[h]
                nc.sync.dma_start(
                    out=bt2,
                    in_=bass.AP(
                        tensor=h_ap.tensor,
                        offset=h_ap.offset,
                        ap=[h_ap.ap[0], [P * BWID, 2], [1, BWID]],
                    ),
                )
                bt2_cur[0] = bt2
            bt = bt2_cur[0][:, (h % 2) * BWID : (h % 2 + 1) * BWID]
            psA_ = psA.tile([P, 1536], fp32, tag="psA")
            # packed into 3 PSUM banks so every matmul output stays inside
            # one bank and a SINGLE exp covers [0:1280]:
            #   bank0: j0 t[0:512]; bank1: j1 t[128:512] | j3 t[384:512];
            #   bank2: j2 t[256:512]
            nc.tensor.matmul(
                psA_[:, 0:512], lhsT=kh[:, 0:P], rhs=qh[:, 0:],
                start=True, stop=True,
            )
            nc.tensor.matmul(
                psA_[:, 512:896], lhsT=kh[:, P : 2 * P], rhs=qh[:, P :],
                start=True, stop=True,
            )
            nc.tensor.matmul(
                psA_[:, 896:1024], lhsT=kh[:, 3 * P : 4 * P], rhs=qh[:, 3 * P :],
                start=True, stop=True,
            )
            nc.tensor.matmul(
                psA_[:, 1024:1280], lhsT=kh[:, 2 * P : 3 * P], rhs=qh[:, 2 * P :],
                start=True, stop=True,
            )
            stage[i] = dict(psA=psA_, bt=bt)

        def emit_exp_mul(i):
            st = stage[i]
            ptA = soft.tile([P, BWID], bf16, tag="ptA")
            nc.scalar.activation(out=ptA, in_=st["psA"][:, 0:BWID], func=Act.Exp)
            pmA = soft.tile([P, BWID], bf16, tag="pmA")
            nc.vector.tensor_mul(out=pmA, in0=ptA, in1=st["bt"])
            st.update(pmA=pmA)

        def emit_AV(i):
            b, h = heads[i]
            st = stage[i]
            pv = psO.tile([D + 1, 512], fp32, tag="psO")

            def vb(j):
                return vaug[:, b * TB + j, h * (D + 1) : (h + 1) * (D + 1)]

            nc.tensor.matmul(
                pv[:, 0:512], lhsT=vb(0), rhs=st["pmA"][:, 0:512],
                start=True, stop=False, skip_group_check=True,
            )
            nc.tensor.matmul(
                pv[:, 128:512], lhsT=vb(1), rhs=st["pmA"][:, 512:896],
                start=False, stop=False, skip_group_check=True,
            )
            nc.tensor.matmul(
                pv[:, 256:512], lhsT=vb(2), rhs=st["pmA"][:, 1024:1280],
                start=False, stop=False, skip_group_check=True,
            )
            nc.tensor.matmul(
                pv[:, 384:512], lhsT=vb(3), rhs=st["pmA"][:, 896:1024],
                start=False, stop=True, skip_group_check=True,
            )
            if h == 0:
                rc = small.tile([P, 4 * T], bf16, tag="rcol", bufs=2)
                rcol[b] = rc
            # head h lives at partition 32*(h%4) (quadrant starts only),
            # free offset (h//4)*T
            with nc.allow_low_precision("softmax 1/l in bf16"):
                nc.vector.reciprocal(
                    out=rcol[b][32 * (h % 4) : 32 * (h % 4) + 1,
                                (h // 4) * T : (h // 4 + 1) * T],
                    in_=pv[D : D + 1, :],
                )
            mo, po = h // 2, (h % 2) * D
            # copy the (unnormalized) AV block out of PSUM right away so
            # the psO bank frees after this pipeline step
            nc.vector.tensor_copy(
                out=oT_bf[po : po + D, mo, b * T : (b + 1) * T],
                in_=pv[0:D, :],
            )
            del stage[i]

        rd = r_dram[:]
        rcol = {}

        def emit_norm_finish(b):
            for q in range(4):
                nc.sync.dma_start(
                    out=bass.AP(
                        tensor=rd.tensor,
                        offset=rd.offset + b * H * T + q * T,
                        ap=[[0, 1], [4 * T, 4], [1, T]],
                    ),
                    in_=rcol[b][32 * q : 32 * q + 1, :],
                )
            r_bc = soft.tile([P, KO, T], bf16, tag="r_bc", bufs=2)
            for half in range(2):
                nc.sync.dma_start(
                    out=r_bc[half * D : (half + 1) * D],
                    in_=bass.AP(
                        tensor=rd.tensor,
                        offset=rd.offset + b * H * T + half * T,
                        ap=[[0, D], [2 * T, KO], [1, T]],
                    ),
                )
            nc.vector.tensor_mul(
                out=oT_bf[:, :, b * T : (b + 1) * T],
                in0=oT_bf[:, :, b * T : (b + 1) * T],
                in1=r_bc,
            )

        if skip_attn:
            nc.vector.memset(oT_bf[:], 0.01)
            fill.drain_gated_at_most(NHEADS + 2)
        else:
            pre = 0  # filler before S delays exp's producer; keep all after
            for i in range(NHEADS + 2):
                if i == H + 1:
                    # all V (seq 1) groups must precede the first b=1 AV
                    fill.drain_gated_at_most(0)
                # filler before S covers the psA WAR wait on exp(i-1)
                fill.emit(i, pre)
                if i < NHEADS:
                    emit_S(i)
                if 0 <= i - 2 < NHEADS and heads[i - 2][1] == H - 1:
                    emit_norm_finish(heads[i - 2][0])
                fill.emit(i, fill_per_head - pre)
                if 0 <= i - 1 < NHEADS:
                    emit_AV(i - 1)
                if i < NHEADS:
                    emit_exp_mul(i)
            fill.drain_gated_at_most(NHEADS + 2)

    if split_waits:
        _split_big_waits(nc, mybir, limit=1)
    return nc


def make_in_maps(inputs):
    import ml_dtypes

    bf = ml_dtypes.bfloat16
    x = np.asarray(inputs["x"], dtype=np.float32)
    scale = np.float32(1.0 / np.sqrt(D))
    wT = {}
    for k in "qkvo":
        w = np.asarray(inputs[f"W{k}"], dtype=np.float32).T  # [c_in, c_out]
        if k == "q":
            w = w * scale  # exact power-of-two scale
        wT[k] = np.ascontiguousarray(w.astype(bf))
    bq = np.asarray(inputs["bq"], dtype=np.float32) * scale
    bk = np.asarray(inputs["bk"], dtype=np.float32)
    bvb = np.asarray(inputs["bv"], dtype=np.float32).astype(bf)
    bob = np.asarray(inputs["bo"], dtype=np.float32).astype(bf)

    # multiplicative bias: exp(S+bias) = exp(S)*exp(bias); causal mask is
    # an exact multiplicative zero.  [h, s, t] layout packed per head into
    # rows of 1280: j0 t[0:512] | j1 t[128:512] | j2 t[256:512] | j3 t[384:512]
    bm = np.asarray(inputs["rel_pos_bias"], dtype=np.float32)[:, :T, :T].copy()
    iu = np.triu_indices(T, 1)
    bm[:, iu[0], iu[1]] = NEG
    eb = np.exp(bm.transpose(0, 2, 1))  # [h, s, t]
    pk = np.empty((H, P, BWID), np.float32)
    pk[:, :, 0:512] = eb[:, 0:128, 0:512]
    pk[:, :, 512:896] = eb[:, 128:256, 128:512]
    pk[:, :, 896:1024] = eb[:, 384:512, 384:512]
    pk[:, :, 1024:1280] = eb[:, 256:384, 256:512]
    pk = np.ascontiguousarray(pk.astype(bf))

    xT_all = x.reshape(N_CORES, NT, C).transpose(0, 2, 1)
    in_maps = []
    for c in range(N_CORES):
        in_maps.append(
            {
                "xT": np.ascontiguousarray(xT_all[c]).astype(bf),
                "wqT": wT["q"],
                "wkT": wT["k"],
                "wvT": wT["v"],
                "woT": wT["o"],
                "bq": bq,
                "bk": bk,
                "bvb": bvb,
                "bob": bob,
                "bmp": pk,
            }
        )
    return in_maps


def build_jitted(nc, n_cores=N_CORES):
    """Build a persistent jitted shard_map executable for `nc` (the
    multi-core path of bass2jax.run_bass_via_pjrt, kept resident so repeat
    kernel() calls skip retracing)."""
    import jax
    from jax.experimental.shard_map import shard_map
    from jax.sharding import Mesh, NamedSharding, PartitionSpec

    from concourse import mybir
    from concourse.bass2jax import (
        _bass_exec_p,
        install_neuronx_cc_hook,
        partition_id_tensor,
    )

    install_neuronx_cc_hook()
    partition_name = nc.partition_id_tensor.name if nc.partition_id_tensor else None

    in_names, out_names, out_avals, zero_outs = [], [], [], []
    for alloc in nc.m.functions[0].allocations:
        if not isinstance(alloc, mybir.MemoryLocationSet):
            continue
        name = alloc.memorylocations[0].name
        if alloc.kind == "ExternalInput":
            if name != partition_name:
                in_names.append(name)
        elif alloc.kind == "ExternalOutput":
            out_names.append(name)
            shape = tuple(alloc.tensor_shape)
            dtype = mybir.dt.np(alloc.dtype)
            out_avals.append(jax.core.ShapedArray(shape, dtype))
            zero_outs.append(np.zeros(shape, dtype))
    n_params = len(in_names)
    n_outs = len(out_avals)
    all_in_names = list(in_names) + list(out_names)
    if partition_name is not None:
        all_in_names.append(partition_name)
    donate = tuple(range(n_params, n_params + n_outs))

    def _body(*args):
        operands = list(args)
        if partition_name is not None:
            operands.append(partition_id_tensor())
        outs = _bass_exec_p.bind(
            *operands,
            out_avals=tuple(out_avals),
            in_names=tuple(all_in_names),
            out_names=tuple(out_names),
            lowering_input_output_aliases=(),
            sim_require_finite=True,
            sim_require_nnan=True,
            nc=nc,
        )
        return tuple(outs)

    devices = jax.devices()[:n_cores]
    mesh = Mesh(np.asarray(devices), ("core",))
    in_specs = (PartitionSpec("core"),) * (n_params + n_outs)
    out_specs = (PartitionSpec("core"),) * n_outs
    jitted = jax.jit(
        shard_map(_body, mesh=mesh, in_specs=in_specs, out_specs=out_specs,
                  check_rep=False),
        donate_argnums=donate,
        keep_unused=True,
    )
    sharding = NamedSharding(mesh, PartitionSpec("core"))
    return jitted, in_names, out_names, out_avals, zero_outs, sharding


def get_runner():
    """Build the program + executable once; return in_maps -> per-core
    output dicts."""
    if "runner" in _CACHE:
        return _CACHE["runner"]
    import jax

    nc = build_program()
    jitted, in_names, out_names, out_avals, zero_outs, sharding = build_jitted(nc)
    n_cores = N_CORES

    def runner(in_maps):
        concat_in = [
            jax.device_put(
                np.concatenate(
                    [np.asarray(in_maps[c][nm]) for c in range(n_cores)], axis=0
                ),
                sharding,
            )
            for nm in in_names
        ]
        zeros = [
            jax.device_put(
                np.zeros((n_cores * z.shape[0], *z.shape[1:]), z.dtype), sharding
            )
            for z in zero_outs
        ]
        out_arrs = jitted(*concat_in, *zeros)
        return [
            {
                nm: np.asarray(out_arrs[i]).reshape(n_cores, *out_avals[i].shape)[c]
                for i, nm in enumerate(out_names)
            }
            for c in range(n_cores)
        ]

    _CACHE["runner"] = runner
    _CACHE["nc"] = nc
    return runner


def kernel(**inputs) -> np.ndarray:
    runner = get_runner()
    in_maps = make_in_maps(inputs)
    results = runner(in_maps)
    out = np.concatenate(
        [results[c]["y"].reshape(B_LOC, T, C) for c in range(N_CORES)], axis=0
    )
    return out.astype(np.float32)
